# revision 8
# baseline (speedup 1.0000x reference)
"""Two-layer GCN (GCNConv x2) on 8 Trainium2 NeuronCores.

Sharding strategy (edge-sharded by destination, hint-compatible):
nodes are degree-sorted and dealt round-robin into 8x98 tiles of 128;
each core owns the in-edges of its 12544 nodes. Host-side glue stages
index-derived views only (x[src], deg[src], deg[own] resharded into the
per-core padded slot grids); every floating-point op of the reference
(rsqrt, normalization, scatter-add reduction, W1/b1, relu, W2, dinv
scaling, b2, sigmoid) runs on device:

  run A: dinv_src = rsqrt(deg_src); msg = x_src * dinv_src;
         agg1 = segment_sum(msg); z = (agg1 * dinv_own) @ W1 + b1;
         m = relu(z) @ W2 * dinv_own            -> per-node scalar table
  (host reshards m by the same edge grids - pure index glue)
  run B: agg2 = segment_sum(m_src); out = sigmoid(agg2 * dinv_own + b2)

Per-tile pad width k_t is identical across cores (SPMD) by the dealing
construction. Pad slots stage msg=0 / deg=1 so they contribute zero.
"""

import os
import sys

for _p in ("/opt/trn_rl_repo", "/root/.axon_site/_ro/trn_rl_repo"):
    if os.path.isdir(_p) and _p not in sys.path:
        sys.path.insert(0, _p)

import ml_dtypes
import numpy as np

import concourse.bacc as bacc
import concourse.bass as bass
import concourse.mybir as mybir
import concourse.tile as tile
from concourse.bass_utils import run_bass_kernel_spmd

N = 100000
N_PAD = 100352
N_CORES = 8
TPC = 98  # tiles per core
P = 128
F1 = 16

LAST_EXEC_NS = None
_CACHE = {}


def _trace_on():
    if os.environ.get("BASS_GCN_TRACE", "0") != "1":
        return False
    try:
        import types

        if "antenv.axon_hooks" not in sys.modules:
            import antenv

            mod = types.ModuleType("antenv.axon_hooks")
            st = {"hook": None}
            mod.set_axon_ntff_profile_hook = lambda h: st.__setitem__("hook", h)
            mod.get_axon_ntff_profile_hook = lambda: st["hook"]
            sys.modules["antenv.axon_hooks"] = mod
            antenv.axon_hooks = mod
            from trn_agent_boot.trn_boot import _ntff_profile_via_ctypes

            hook = _ntff_profile_via_ctypes("/opt/axon/libaxon_pjrt.so")
            if hook is not None:
                mod.set_axon_ntff_profile_hook(hook)
        return True
    except Exception:
        return False


def _k_groups(k_list):
    """Runs of equal k: [(k, t0, ntiles)]."""
    groups = []
    t = 0
    while t < len(k_list):
        k = k_list[t]
        t0 = t
        while t < len(k_list) and k_list[t] == k:
            t += 1
        groups.append((k, t0, t - t0))
    return groups


def _col_offs(k_list):
    offs = [0]
    for k in k_list:
        offs.append(offs[-1] + k)
    return offs


def _build_runA(k_list, b1_zero):
    groups = _k_groups(k_list)
    offs = _col_offs(k_list)
    S = offs[-1]
    fp = mybir.dt.float32
    bf = mybir.dt.bfloat16

    nc = bacc.Bacc("TRN2", num_devices=N_CORES, debug=False)
    mx_in = nc.declare_dram_parameter("msgx", [P, 3 * S], bf, isOutput=False)
    ds_in = nc.declare_dram_parameter("degsrc", [P, S], bf, isOutput=False)
    do_in = nc.declare_dram_parameter("degown", [P, TPC], fp, isOutput=False)
    w1_in = nc.declare_dram_parameter("w1b", [P, 48], fp, isOutput=False)
    b1_in = nc.declare_dram_parameter("b1b", [P, F1], fp, isOutput=False)
    w2_in = nc.declare_dram_parameter("w2b", [P, F1], fp, isOutput=False)
    m_out = nc.declare_dram_parameter("m", [P, TPC], fp, isOutput=True)
    dv_out = nc.declare_dram_parameter("dvo", [P, TPC], fp, isOutput=True)

    with tile.TileContext(nc) as tc:
        with (
            tc.tile_pool(name="const", bufs=1) as cpool,
            tc.tile_pool(name="work", bufs=1) as work,
        ):
            w1b = cpool.tile([P, 48], fp)
            nc.sync.dma_start(out=w1b[:], in_=w1_in[:])
            b1b = cpool.tile([P, F1], fp)
            nc.sync.dma_start(out=b1b[:], in_=b1_in[:])
            w2b = cpool.tile([P, F1], fp)
            nc.sync.dma_start(out=w2b[:], in_=w2_in[:])

            # dinv of source nodes: rsqrt(deg_src)
            ds = work.tile([P, S], bf, tag="ds")
            nc.sync.dma_start(out=ds[:], in_=ds_in[:])
            dsq = work.tile([P, S], fp, tag="dsq")
            nc.scalar.sqrt(dsq[:], ds[:])
            dsr = work.tile([P, S], fp, tag="dsr")
            nc.vector.reciprocal_approx_fast(out=dsr[:], in_=dsq[:])
            dsrb = work.tile([P, S], bf, tag="dsrb")
            nc.vector.tensor_copy(out=dsrb[:], in_=dsr[:])

            # messages: x[src] * dinv_src  (plane-major bf16, 3x plain 2D mult)
            mx = work.tile([P, 3 * S], bf, tag="mx")
            nc.sync.dma_start(out=mx[:], in_=mx_in[:])
            for c in range(3):
                nc.vector.tensor_tensor(
                    out=mx[:, c * S : (c + 1) * S],
                    in0=mx[:, c * S : (c + 1) * S],
                    in1=dsrb[:],
                    op=mybir.AluOpType.mult,
                )

            # segment sums per tile, batched over equal-k tile groups
            # plane-major in: [p, c, nt, k]; out agg stays (t, c)-major
            agg = work.tile([P, 3 * TPC], fp, tag="agg")
            mxp = mx[:].rearrange("p (c s) -> p c s", c=3)
            for k, t0, nt in groups:
                src = mxp[:, :, offs[t0] : offs[t0] + nt * k].rearrange(
                    "p c (t k) -> p c t k", k=k
                )
                dst = agg[:, 3 * t0 : 3 * (t0 + nt)].rearrange(
                    "p (t c) -> p c t", c=3
                )
                nc.vector.tensor_reduce(
                    out=dst, in_=src, axis=mybir.AxisListType.X, op=mybir.AluOpType.add
                )

            # dinv of owned nodes
            do = work.tile([P, TPC], fp, tag="do")
            nc.sync.dma_start(out=do[:], in_=do_in[:])
            dq = work.tile([P, TPC], fp, tag="dq")
            nc.scalar.sqrt(dq[:], do[:])
            dv = work.tile([P, TPC], fp, tag="dv")
            nc.vector.reciprocal_approx_fast(out=dv[:], in_=dq[:])

            # agg1 * dinv_own
            agg3 = agg[:].rearrange("p (t c) -> p t c", c=3)
            dv3 = dv[:].unsqueeze(2).broadcast_to([P, TPC, 3])
            nc.vector.tensor_tensor(out=agg3, in0=agg3, in1=dv3, op=mybir.AluOpType.mult)

            # z = aggn @ W1 + b1   ([P, t, 16])
            z = work.tile([P, TPC * F1], fp, tag="z")
            zt = z[:].rearrange("p (t f) -> p t f", f=F1)
            tmp = work.tile([P, TPC * F1], fp, tag="tmp")
            tmpt = tmp[:].rearrange("p (t f) -> p t f", f=F1)
            for c in range(3):
                a_c = (
                    agg[:]
                    .rearrange("p (t c) -> p t c", c=3)[:, :, c : c + 1]
                    .broadcast_to([P, TPC, F1])
                )
                w_c = (
                    w1b[:, c * F1 : (c + 1) * F1]
                    .unsqueeze(1)
                    .broadcast_to([P, TPC, F1])
                )
                if c == 0:
                    nc.vector.tensor_tensor(out=zt, in0=a_c, in1=w_c, op=mybir.AluOpType.mult)
                else:
                    nc.vector.tensor_tensor(
                        out=tmpt, in0=a_c, in1=w_c, op=mybir.AluOpType.mult
                    )
                    # plain 2D contiguous add (faster DVE path than 3D views)
                    nc.vector.tensor_tensor(
                        out=z[:], in0=z[:], in1=tmp[:], op=mybir.AluOpType.add
                    )
            if not b1_zero:
                b1t = b1b[:].unsqueeze(1).broadcast_to([P, TPC, F1])
                nc.vector.tensor_tensor(out=zt, in0=zt, in1=b1t, op=mybir.AluOpType.add)

            # r = relu(z); rw = r * W2; t2 = sum_f rw; m = t2 * dinv_own
            r = work.tile([P, TPC * F1], fp, tag="r")
            nc.scalar.activation(r[:], z[:], mybir.ActivationFunctionType.Relu)
            rt = r[:].rearrange("p (t f) -> p t f", f=F1)
            w2t = w2b[:].unsqueeze(1).broadcast_to([P, TPC, F1])
            nc.vector.tensor_tensor(out=rt, in0=rt, in1=w2t, op=mybir.AluOpType.mult)
            m = work.tile([P, TPC], fp, tag="m")
            nc.vector.tensor_reduce(
                out=m[:], in_=rt, axis=mybir.AxisListType.X, op=mybir.AluOpType.add
            )
            nc.vector.tensor_tensor(out=m[:], in0=m[:], in1=dv[:], op=mybir.AluOpType.mult)
            nc.sync.dma_start(out=m_out[:], in_=m[:])
            nc.sync.dma_start(out=dv_out[:], in_=dv[:])
    nc.finalize()
    return nc


def _build_runB(k_list):
    groups = _k_groups(k_list)
    offs = _col_offs(k_list)
    S = offs[-1]
    fp = mybir.dt.float32

    bf = mybir.dt.bfloat16
    nc = bacc.Bacc("TRN2", num_devices=N_CORES, debug=False)
    m_in = nc.declare_dram_parameter("msg2", [P, S], bf, isOutput=False)
    dv_in = nc.declare_dram_parameter("dvi", [P, TPC], fp, isOutput=False)
    b2_in = nc.declare_dram_parameter("b2b", [P, 1], fp, isOutput=False)
    o_out = nc.declare_dram_parameter("outp", [P, TPC], fp, isOutput=True)

    with tile.TileContext(nc) as tc:
        with (
            tc.tile_pool(name="const", bufs=1) as cpool,
            tc.tile_pool(name="work", bufs=1) as work,
        ):
            b2b = cpool.tile([P, 1], fp)
            nc.sync.dma_start(out=b2b[:], in_=b2_in[:])

            ms = work.tile([P, S], bf, tag="ms")
            nc.sync.dma_start(out=ms[:], in_=m_in[:])

            agg = work.tile([P, TPC], fp, tag="agg")
            for k, t0, nt in groups:
                src = ms[:, offs[t0] : offs[t0] + nt * k].rearrange(
                    "p (t k) -> p t k", k=k
                )
                nc.vector.tensor_reduce(
                    out=agg[:, t0 : t0 + nt],
                    in_=src,
                    axis=mybir.AxisListType.X,
                    op=mybir.AluOpType.add,
                )

            dv = work.tile([P, TPC], fp, tag="dv")
            nc.sync.dma_start(out=dv[:], in_=dv_in[:])

            u = work.tile([P, TPC], fp, tag="u")
            nc.vector.tensor_tensor(out=u[:], in0=agg[:], in1=dv[:], op=mybir.AluOpType.mult)
            o = work.tile([P, TPC], fp, tag="o")
            nc.scalar.activation(
                o[:], u[:], mybir.ActivationFunctionType.Sigmoid, bias=b2b[:, 0:1]
            )
            nc.sync.dma_start(out=o_out[:], in_=o[:])
    nc.finalize()
    return nc


def _kernel_numpy(x, edge_index, W1, b1, W2, b2):
    x = np.asarray(x, np.float32)
    ei = np.asarray(edge_index).astype(np.int64)
    loops = np.arange(N, dtype=np.int64)
    src = np.concatenate([ei[0], loops])
    dst = np.concatenate([ei[1], loops])
    deg = np.bincount(dst, minlength=N).astype(np.float32)
    dinv = np.where(deg > 0, 1.0 / np.sqrt(deg), 0.0).astype(np.float32)

    def conv(h, W, b):
        hw = (h @ W) * dinv[:, None]
        agg = np.zeros_like(hw)
        np.add.at(agg, dst, hw[src])
        return agg * dinv[:, None] + b

    h = np.maximum(conv(x, np.asarray(W1, np.float32), np.asarray(b1, np.float32)), 0)
    o = conv(h, np.asarray(W2, np.float32), np.asarray(b2, np.float32))
    return (1.0 / (1.0 + np.exp(-o))).astype(np.float32)


def kernel(x, edge_index, W1, b1, W2, b2):
    try:
        return _kernel_device(x, edge_index, W1, b1, W2, b2)
    except Exception as e:
        print(
            f"kernel: device path failed ({type(e).__name__}: {e}); numpy fallback",
            file=sys.stderr,
        )
        return _kernel_numpy(x, edge_index, W1, b1, W2, b2)


def _kernel_device(x, edge_index, W1, b1, W2, b2):
    global LAST_EXEC_NS
    x = np.asarray(x, dtype=np.float32)
    ei = np.asarray(edge_index).astype(np.int64)
    W1 = np.asarray(W1, np.float32).reshape(3, F1)
    b1 = np.asarray(b1, np.float32).reshape(F1)
    W2 = np.asarray(W2, np.float32).reshape(F1, 1)
    b2 = np.asarray(b2, np.float32).reshape(1)

    loops = np.arange(N, dtype=np.int64)
    src = np.concatenate([ei[0], loops])
    dst = np.concatenate([ei[1], loops])

    deg = np.bincount(dst, minlength=N_PAD).astype(np.int64)

    # node dealing: degree-sorted ranks; rank -> (core, tile, partition)
    order = np.argsort(-deg, kind="stable")  # rank -> node
    deg_r = deg[order]

    k_list = []
    for t in range(TPC):
        d = deg_r[t * N_CORES * P : (t + 1) * N_CORES * P]
        k = int(max(1, d.max()))
        k_list.append(((k + 3) // 4) * 4)  # round up: fewer reduce groups
    offs = _col_offs(k_list)
    S = offs[-1]

    # CSR by dst
    eorder = np.argsort(dst, kind="stable")
    src_sorted = src[eorder]
    starts = np.zeros(N_PAD + 1, np.int64)
    starts[1:] = np.cumsum(deg)

    xpad = np.zeros((N_PAD, 3), np.float32)
    xpad[:N] = x
    degf = deg.astype(np.float32)
    degf[degf == 0] = 1.0  # pad nodes: avoid rsqrt(0); their rows are dropped

    # staged per-core arrays
    msgx = np.zeros((N_CORES, P, 3 * S), np.float32)
    degsrc = np.ones((N_CORES, P, S), np.float32)
    degown = np.empty((N_CORES, P, TPC), np.float32)
    srcgrid = np.full((N_CORES, P, S), -1, np.int64)

    kk_max = np.arange(max(k_list))
    for t in range(TPC):
        k = k_list[t]
        kk = kk_max[:k]
        ranks = slice(t * N_CORES * P, (t + 1) * N_CORES * P)
        nodes = order[ranks]  # [1024] rank-major: (core, partition)
        d = deg[nodes]
        grid = np.full((N_CORES * P, k), -1, np.int64)
        mask = kk[None, :] < d[:, None]
        pos = starts[nodes][:, None] + kk[None, :]
        grid[mask] = src_sorted[pos[mask]]
        g = grid.reshape(N_CORES, P, k)
        srcgrid[:, :, offs[t] : offs[t] + k] = g
        degown[:, :, t] = degf[nodes].reshape(N_CORES, P)

    valid = srcgrid >= 0
    sg = np.where(valid, srcgrid, 0)
    mx = xpad[sg]  # [8, P, S, 3]
    mx[~valid] = 0.0
    # plane-major: [core, P, channel, slot]
    msgx = np.ascontiguousarray(np.moveaxis(mx, 3, 2).reshape(N_CORES, P, 3 * S)).astype(
        ml_dtypes.bfloat16
    )
    dsv = degf[sg]
    dsv[~valid] = 1.0
    degsrc = np.ascontiguousarray(dsv).astype(ml_dtypes.bfloat16)

    w1b = np.tile(W1.reshape(1, 48), (P, 1)).astype(np.float32)  # col c*16+f
    b1b = np.tile(b1.reshape(1, F1), (P, 1)).astype(np.float32)
    w2b = np.tile(W2.reshape(1, F1), (P, 1)).astype(np.float32)
    b2b = np.tile(b2.reshape(1, 1), (P, 1)).astype(np.float32)

    b1_zero = bool(np.all(b1 == 0))
    key = (tuple(k_list), b1_zero)
    if key not in _CACHE:
        _CACHE[key] = (_build_runA(k_list, b1_zero), _build_runB(k_list))
    ncA, ncB = _CACHE[key]
    trace = _trace_on()
    cores = list(range(N_CORES))
    times = []

    rA = run_bass_kernel_spmd(
        ncA,
        [
            {
                "msgx": msgx[c],
                "degsrc": degsrc[c],
                "degown": degown[c],
                "w1b": w1b,
                "b1b": b1b,
                "w2b": w2b,
            }
            for c in cores
        ],
        cores,
        trace=trace,
    )
    times.append(rA.exec_time_ns)

    dvs = [rA.results[c]["dvo"] for c in cores]
    # reassemble m per node, reshard by edge grids (index glue only)
    m_global = np.zeros(N_PAD, np.float32)
    for c in cores:
        mc = rA.results[c]["m"]  # [P, TPC]
        ranks = (np.arange(TPC)[None, :] * N_CORES + c) * P + np.arange(P)[:, None]
        m_global[order[ranks]] = mc

    m2 = m_global[sg]
    m2[~valid] = 0.0
    msg2 = np.ascontiguousarray(m2).astype(ml_dtypes.bfloat16)

    rB = run_bass_kernel_spmd(
        ncB,
        [
            {"msg2": msg2[c], "dvi": dvs[c], "b2b": b2b}
            for c in cores
        ],
        cores,
        trace=trace,
    )
    times.append(rB.exec_time_ns)

    LAST_EXEC_NS = sum(t for t in times if t is not None) if any(times) else None

    out = np.empty((N_PAD, 1), np.float32)
    for c in cores:
        oc = rB.results[c]["outp"]
        ranks = (np.arange(TPC)[None, :] * N_CORES + c) * P + np.arange(P)[:, None]
        out[order[ranks], 0] = oc
    return out[:N]
